# revision 37
# baseline (speedup 1.0000x reference)
"""Trainium2 Bass kernel: GQA attention block (QKV proj + RMSNorm + RoPE +
bidirectional attention + output proj), 8 cores.

Sharding: 8 cores = 4 batches x 2 token halves. Each core owns 512 tokens of
one batch: it computes K/V projections for its OWN 512 tokens only, the pair
of cores sharing a batch AllGathers K/V (2MB per core, hidden under the Q
projection), then each core runs attention + o_proj for its 512 query tokens.

Per-core phases (all matmuls bf16, fp32 accumulation):
  P1  K/V proj for own 512 tokens, RMSNorm+RoPE on K, PE-transpose K.
      Store K^T/V shard to DRAM, pairwise AllGather, reload both halves.
  P2a Q projection (all 32 heads, own 512 tokens), RMSNorm+RoPE,
      PE-transpose into qT [d, h, q]. Collective + K/V reload overlap here.
  P2b Attention, software-pipelined across heads: scores(h) interleaved with
      av(h-1) on the PE; exp on ScalarE; Z-tree (DVE) one head behind;
      partition reduce (Pool); 1/Z + aT scale (DVE) two heads behind.
  P3  o_proj: y = aT.T @ woT, fp32 out.

Weights are host-retiled so each chunk loads as 4 x 512KB contiguous DMAs
(the v1 32x128KB pattern saturated the SP sequencer's ~600ns/DMA issue rate).
"""

import os
import sys
from contextlib import ExitStack

for _p in (
    "/root/.axon_site",
    "/root/.axon_site/_ro/trn_rl_repo",
    "/root/.axon_site/_ro/pypackages",
    "/opt/trn_rl_repo",
):
    if os.path.isdir(_p) and _p not in sys.path:
        sys.path.append(_p)

import ml_dtypes
import numpy as np

import concourse.bacc as bacc
import concourse.bass as bass
import concourse.tile as tile
from concourse import bass_isa, mybir
from concourse.bass_utils import run_bass_kernel_spmd
from concourse.masks import make_identity

BF16 = mybir.dt.bfloat16
F32 = mybir.dt.float32
AF = mybir.ActivationFunctionType
OP = mybir.AluOpType
AX = mybir.AxisListType

B = 4
S = 1024
SQ = 512            # tokens per core (queries AND its K/V shard)
HIDDEN = 4096
NH = 32
NKV = 8
HD = 128
EPS = 1e-6
ROPE_BASE = 1000000.0
SCALE = float(HD) ** -0.5
NDT = HIDDEN // 128  # 32 contraction tiles
N_CORES = 8

_BF = ml_dtypes.bfloat16


def _bcast_mid(ap, n):
    """[P, X...] -> [P, n, X...] with a stride-0 middle dim."""
    return bass.AP(tensor=ap.tensor, offset=ap.offset, ap=[ap.ap[0], [0, n], *ap.ap[1:]])


def build_bass() -> bass.Bass:
    nc = bacc.Bacc("TRN2", target_bir_lowering=False, debug=False, num_devices=N_CORES)

    # DRAM I/O (per core). hs pre-arranged per token-tile [tt, p, a, d];
    # weights pre-tiled [chunk, sub, p, a, col] so each sub loads as one
    # 512KB DMA with 8KB-contiguous partition rows.
    hs_d = nc.declare_dram_parameter("hs_d", [4, 128, NDT, 128], BF16, isOutput=False)
    wkv_t = nc.declare_dram_parameter("wkv_t", [4, 4, 128, 8, 512], BF16, isOutput=False)
    wq_t = nc.declare_dram_parameter("wq_t", [8, 4, 128, 8, 512], BF16, isOutput=False)
    wo_t = nc.declare_dram_parameter("wo_t", [8, 4, 128, 8, 512], BF16, isOutput=False)
    # rope tables [t, cA|sA|cB|sB] (cos/sin with rms-norm weight folded in)
    ropeq = nc.declare_dram_parameter("ropeq", [SQ, 256], F32, isOutput=False)
    ropek = nc.declare_dram_parameter("ropek", [SQ, 256], F32, isOutput=False)
    y = nc.declare_dram_parameter("y", [SQ, HIDDEN], F32, isOutput=True)

    with ExitStack() as ctx:
        tc = ctx.enter_context(tile.TileContext(nc))

        persist = ctx.enter_context(tc.tile_pool(name="persist", bufs=1))
        hs_all = persist.tile([128, 4, NDT, 128], BF16, tag="hs_all")  # 32KB/p
        qT = persist.tile([128, NH, SQ], BF16, tag="qT")          # [d, h, q] 32KB
        ktT = persist.tile([128, NKV, S], BF16, tag="ktT")        # [d, kvh, t] 16KB
        v_all = persist.tile([128, 8, NKV, 128], BF16, tag="v")   # [t%128, tt, kvh, d]
        tabq = persist.tile([128, 4, 256], F32, tag="tabq")
        tabk = persist.tile([128, 4, 256], F32, tag="tabk")
        ident = persist.tile([128, 128], BF16, tag="ident")
        ones = persist.tile([128, 128], BF16, tag="ones")
        # attention out [d, h, q] reuses hs bytes (dead after P2a)
        aT = hs_all[:].rearrange("p t a d -> p (t a d)").rearrange(
            "p (h q) -> p h q", h=NH)

        wp = ctx.enter_context(tc.tile_pool(name="wp", bufs=5))
        scratch = ctx.enter_context(tc.tile_pool(name="scratch", bufs=2))
        qnp = ctx.enter_context(tc.tile_pool(name="qnp", bufs=6))
        psb_pool = ctx.enter_context(tc.tile_pool(name="psb_pool", bufs=2))
        asm_pool = ctx.enter_context(tc.tile_pool(name="asm_pool", bufs=2))
        ysb = ctx.enter_context(tc.tile_pool(name="ysb", bufs=2))
        dram = ctx.enter_context(tc.tile_pool(name="dram", bufs=1, space="DRAM"))

        # One manual PSUM pool covering all 8 banks. Bank map:
        #   P1/P2a/P3: banks 0-2 = projection accum (rotate x3),
        #              banks 3-5 = transpose staging (bf16 views, rotate x3)
        #   P2b:       banks 0-5 = score regions (2 banks each, ring of
        #              3 per kt-pair), banks 6-7 = av (per-head parity)
        psum = ctx.enter_context(tc.tile_pool(name="psum", bufs=1, space="PSUM"))
        P = psum.tile([128, 8, 512], F32, tag="psum_all")

        def bank(i):
            return P[:, i, :]

        # collective bounce buffers: [kt | v] shard, 2MB in, 4MB gathered
        kv_sh = dram.tile([2, 128, 4096], BF16, tag="kv_sh")
        kv_full = dram.tile([2, 2, 128, 4096], BF16, tag="kv_full")

        def load_w_subs(wsrc, c, subs):
            tiles = []
            for sub in subs:
                wt = wp.tile([128, 8, 512], BF16, tag="wt")
                nc.sync.dma_start(out=wt[:], in_=wsrc[c, sub])
                tiles.append(wt)
            return tiles

        def load_w_chunk(wsrc, c):
            """4 [128, 8, 512] tiles covering one 512-col chunk, 512KB DMAs."""
            return load_w_subs(wsrc, c, range(4))

        def wslice(wts, a):
            return wts[a >> 3][:, a & 7, :]

        # First weight chunk + first hs token-tile gate the PE start; hs is
        # loaded per token-tile (1MB each) on the ACT HWDGE ring. (Splitting
        # these into smaller DMAs was tried and HURT: per-queue semaphore
        # slots serialize at ~0.8us/DMA; putting c0s0 on the scalar ring
        # ahead of hs also HURT, +7.6us startup in v11.)
        wts_first = load_w_chunk(wkv_t, 0)
        for tt in range(2):
            nc.scalar.dma_start(out=hs_all[:, tt, :, :], in_=hs_d[tt])
        for tt in range(2, 4):
            # tt2/tt3 ride the (empty until the collective) Pool queue:
            # spreads the ~8.5MB startup burst over a third DMA queue
            nc.gpsimd.dma_start(out=hs_all[:, tt, :, :], in_=hs_d[tt])
        make_identity(nc, ident[:])
        nc.vector.memset(ones[:], 1.0)
        nc.scalar.dma_start(out=tabq[:], in_=ropeq[:].rearrange("(a p) c -> p a c", p=128))
        nc.scalar.dma_start(out=tabk[:], in_=ropek[:].rearrange("(a p) c -> p a c", p=128))

        # Q-chunk-0 subs 0-1 staged EARLY through the psb_pool buffers
        # (idle until P2b, same shape) on the scalar ring (idle after hs):
        # they land ~30us in, instead of ~P1-end+8us via the wp ring whose
        # buffer WARs only release at P1's last reads (v7/v8 12.6us gap).
        wq0_early = []
        for sub in range(2):
            t = psb_pool.tile([128, 8, 512], BF16, tag="p_sb")
            nc.scalar.dma_start(out=t[:], in_=wq_t[0, sub])
            wq0_early.append(t)

        def norm_rope(ps, tab_tile, tt, qn):
            """RMSNorm + RoPE on a [128 tok, 4 heads, 128] psum projection
            (ps is a [128, 512] PSUM bank AP), into bf16 qn [128, 4, 128]."""
            psv = ps.rearrange("p (h d) -> p h d", h=4)
            qf = scratch.tile([128, 4, 128], F32, tag="qf")
            qsq = scratch.tile([128, 512], BF16, tag="qsq")
            ssq = scratch.tile([128, 4], F32, tag="ssq")
            rr = scratch.tile([128, 4], F32, tag="rr")
            t1 = scratch.tile([128, 4, 64], F32, tag="t1")
            t2 = scratch.tile([128, 4, 64], F32, tag="t2")
            t3 = scratch.tile([128, 4, 64], F32, tag="t1")
            t4 = scratch.tile([128, 4, 64], F32, tag="t2")

            nc.scalar.copy(out=qf[:], in_=psv)
            nc.scalar.activation(out=qsq[:], in_=ps, func=AF.Square)
            nc.vector.reduce_sum(
                out=ssq[:], in_=qsq[:].rearrange("p (h d) -> p h d", h=4), axis=AX.X
            )
            # v = ssq/128 + eps, then r = rsqrt(v) via bit-trick seed + 2 Newton
            # iterations (all-DVE; keeps ScalarE on a single ACT table set).
            vv = scratch.tile([128, 4], F32, tag="vv")
            rt = scratch.tile([128, 4], F32, tag="rt")
            nc.vector.tensor_scalar(out=vv[:], in0=ssq[:], scalar1=1.0 / HD,
                                    scalar2=EPS, op0=OP.mult, op1=OP.add)
            vi = vv[:].bitcast(mybir.dt.int32)
            ri = rr[:].bitcast(mybir.dt.int32)
            nc.vector.tensor_scalar(out=ri, in0=vi, scalar1=1, scalar2=None,
                                    op0=OP.arith_shift_right)
            nc.vector.tensor_scalar(out=ri, in0=ri, scalar1=-1, scalar2=0x5F3759DF,
                                    op0=OP.mult, op1=OP.add)
            for _ in range(2):
                nc.vector.tensor_mul(rt[:], rr[:], rr[:])
                nc.vector.tensor_mul(rt[:], rt[:], vv[:])
                nc.vector.tensor_scalar(out=rt[:], in0=rt[:], scalar1=-0.5,
                                        scalar2=1.5, op0=OP.mult, op1=OP.add)
                nc.vector.tensor_mul(rr[:], rr[:], rt[:])
            for hh in range(4):
                nc.vector.tensor_scalar_mul(qf[:, hh, :], qf[:, hh, :], rr[:, hh:hh + 1])
            q1 = qf[:, :, 0:64]
            q2 = qf[:, :, 64:128]
            cA = _bcast_mid(tab_tile[:, tt, 0:64], 4)
            sA = _bcast_mid(tab_tile[:, tt, 64:128], 4)
            cB = _bcast_mid(tab_tile[:, tt, 128:192], 4)
            sB = _bcast_mid(tab_tile[:, tt, 192:256], 4)
            nc.vector.tensor_mul(t1[:], q1, cA)
            nc.vector.tensor_mul(t2[:], q2, sB)
            nc.vector.tensor_sub(qn[:, :, 0:64], t1[:], t2[:])
            nc.vector.tensor_mul(t3[:], q2, cB)
            nc.vector.tensor_mul(t4[:], q1, sA)
            nc.vector.tensor_add(qn[:, :, 64:128], t3[:], t4[:])

        tp_cnt = [0]

        def transpose4(qn, dst_ap):
            """PE-transpose 4 [128,128] heads of qn into dst_ap [128, 4, 128].
            Stages through a bf16 view of PSUM banks 3-5 (rotating)."""
            tp = bank(3 + tp_cnt[0] % 3).bitcast(BF16)[:, 0:512]
            tp_cnt[0] += 1
            for hh in range(4):
                nc.tensor.transpose(tp[:, hh * 128:(hh + 1) * 128], qn[:, hh, :], ident[:])
            nc.scalar.copy(out=dst_ap, in_=tp.rearrange("p (h t) -> p h t", h=4))

        # Transposes deferred TWO token-tiles behind the matmul stream so the
        # PE never waits for the DVE norm/rope latency chain.
        pend_t = []

        def flush_t(keep):
            while len(pend_t) > keep:
                transpose4(*pend_t.pop(0))

        # ---------------- P1: K/V projections for own 512 tokens ----------
        pp_cnt = [0]

        def pp_bank():
            b = bank(pp_cnt[0] % 3)
            pp_cnt[0] += 1
            return b

        for c in range(4):
            wts = wts_first if c == 0 else load_w_chunk(wkv_t, c)
            for tt in range(4):
                ps = pp_bank()
                for a in range(NDT):
                    nc.tensor.matmul(
                        ps, hs_all[:, tt, a, :], wslice(wts, a),
                        start=(a == 0), stop=(a == NDT - 1),
                    )
                if c < 2:  # K chunk: 4 kv heads c*4..c*4+3
                    kn = qnp.tile([128, 4, 128], BF16, tag="qqn")
                    norm_rope(ps, tabk, tt, kn)
                    flush_t(1)
                    pend_t.append(
                        (kn, ktT[:, c * 4:(c + 1) * 4, tt * 128:(tt + 1) * 128]))
                else:      # V chunk: plain bf16 copy
                    flush_t(0)
                    nc.scalar.copy(
                        out=v_all[:, tt, (c - 2) * 4:(c - 1) * 4, :],
                        in_=ps.rearrange("p (h d) -> p h d", h=4),
                    )

        # subs 2-3 of Q-chunk-0 ride the wp ring while P1's tail drains
        wts_q0 = wq0_early + load_w_subs(wq_t, 0, (2, 3))

        # shard -> DRAM, pairwise AllGather, reload both halves in rank
        # order. All kv traffic rides the Pool queue (idle here) so the
        # weight stream owns the sync ring through the P1/P2a boundary.
        nc.gpsimd.dma_start(
            out=kv_sh[0].rearrange("p (h t) -> p h t", h=NKV),
            in_=ktT[:, :, 0:SQ],
        )
        nc.gpsimd.dma_start(
            out=kv_sh[1].rearrange("p (t k d) -> p t k d", t=4, k=NKV),
            in_=v_all[:, 0:4, :, :],
        )
        nc.gpsimd.collective_compute(
            "AllGather",
            mybir.AluOpType.bypass,
            replica_groups=[[2 * i, 2 * i + 1] for i in range(N_CORES // 2)],
            ins=[kv_sh.opt()],
            outs=[kv_full.opt()],
        )
        for g in range(2):
            nc.gpsimd.dma_start(
                out=ktT[:, :, g * SQ:(g + 1) * SQ],
                in_=kv_full[g, 0].rearrange("p (h t) -> p h t", h=NKV),
            )
            nc.gpsimd.dma_start(
                out=v_all[:, g * 4:(g + 1) * 4, :, :],
                in_=kv_full[g, 1].rearrange("p (t k d) -> p t k d", t=4, k=NKV),
            )

        # ---------------- P2a: Q projection, all 32 heads ------------------
        for c in range(8):
            wts = wts_q0 if c == 0 else load_w_chunk(wq_t, c)
            for qt in range(4):
                ps = pp_bank()
                for a in range(NDT):
                    nc.tensor.matmul(
                        ps, hs_all[:, qt, a, :], wslice(wts, a),
                        start=(a == 0), stop=(a == NDT - 1),
                    )
                qn = qnp.tile([128, 4, 128], BF16, tag="qqn")
                norm_rope(ps, tabq, qt, qn)
                flush_t(1)
                pend_t.append(
                    (qn, qT[:, c * 4:(c + 1) * 4, qt * 128:(qt + 1) * 128]))

        # prefetch first o_proj weight chunk; streams during attention
        wts_o0 = load_w_chunk(wo_t, 0)
        flush_t(0)

        # ---------------- P2b: attention, pipelined across heads -----------
        # kt tiles batched in pairs; each pair: 2 scores MMs into a 2-bank
        # PSUM region (ring of 3 regions over banks 0-5), ONE batched exp
        # ACT ((1024+352)/1.2 ns vs 2x(512+352)/1.2), av(h-1) interleaved.
        # av(h-1) is drained to SBUF by DVE as soon as its 8 MMs finish,
        # releasing the av bank immediately: without this, av(h+1)'s
        # start-MM (WAR on the aT mul) chains through the ~8us Z tail
        # (tree -> Pool partition-reduce 3.5us -> recip -> mul) and
        # head-of-line blocks the in-order PE queue (v3 lesson).
        GROUPS = [(0, 2), (2, 2), (4, 2), (6, 2)]
        pend = {}
        greg = [0]  # global group counter -> cycles score regions

        def finish_h2(h):
            if 0 <= h - 2 < NH:
                e = pend.pop(h - 2)
                rz = asm_pool.tile([128, 512], F32, tag="rz")
                nc.vector.reciprocal_approx_fast(out=rz[:], in_=e["zbc"])
                nc.vector.tensor_mul(aT[:, e["h"], :], e["av_sb"][:], rz[:])

        def drain_av(e):
            """DVE copy av PSUM -> SBUF right after its last MM, so the av
            bank is released without waiting for the Z tail."""
            av_sb = asm_pool.tile([128, 512], F32, tag="av_sb")
            nc.vector.tensor_copy(av_sb[:], e["av"])
            e["av_sb"] = av_sb

        for h in range(NH + 2):
            # NOTE: finish_h2 (recip+mul for h-2) must come AFTER the tree
            # block below in the DVE queue: recip(h-2) waits on the Pool
            # preduce(h-2), whose input is the previous iteration's last
            # DVE op. With recip first, DVE idles through every preduce
            # (serial DVE<->Pool ping-pong, ~+3.4us/head — v5 lesson).
            if h < NH:
                hv = h // 4  # kv head (GQA group of 4)
                p_sb = psb_pool.tile([128, 8, 512], BF16, tag="p_sb")
                av = bank(6 + (h & 1))
                prev = pend.get(h - 1)
                for kt0, n in GROUPS:
                    base = 2 * (greg[0] % 3)
                    greg[0] += 1
                    sc = P[:, base:base + n, :]
                    for j in range(n):
                        nc.tensor.matmul(
                            sc[:, j, :],
                            ktT[:, hv, (kt0 + j) * 128:(kt0 + j + 1) * 128],
                            qT[:, h, :],
                            start=True, stop=True,
                        )
                    nc.scalar.activation(out=p_sb[:, kt0:kt0 + n, :], in_=sc,
                                         func=AF.Exp, scale=SCALE)
                    if prev is not None:  # av kt pair matching this group
                        for kt in range(kt0, kt0 + n):
                            nc.tensor.matmul(
                                prev["av"], v_all[:, kt, prev["hv"], :],
                                prev["p_sb"][:, kt, :],
                                start=(kt == 0), stop=(kt == 7),
                            )
                if prev is not None:
                    drain_av(prev)
                pend[h] = dict(h=h, hv=hv, p_sb=p_sb, av=av)
            elif (h - 1) in pend:  # tail: av for the last head
                prev = pend[h - 1]
                for kt in range(8):
                    nc.tensor.matmul(
                        prev["av"], v_all[:, kt, prev["hv"], :],
                        prev["p_sb"][:, kt, :],
                        start=(kt == 0), stop=(kt == 7),
                    )
                drain_av(prev)
            if 0 <= h - 1 < NH:
                e = pend[h - 1]
                p_sb = e["p_sb"]
                t4 = asm_pool.tile([128, 4, 512], BF16, tag="t4")
                t2 = asm_pool.tile([128, 2, 512], BF16, tag="t2")
                acc = asm_pool.tile([128, 512], BF16, tag="acc")
                nc.vector.tensor_add(t4[:], p_sb[:, 0:4, :], p_sb[:, 4:8, :])
                nc.vector.tensor_add(t2[:], t4[:, 0:2, :], t4[:, 2:4, :])
                nc.vector.tensor_add(acc[:], t2[:, 0, :], t2[:, 1, :])
                # partition-sum of acc via an all-ones matmul (~213ns on the
                # PE, result broadcast to all 128 partitions), written into
                # av(h-1)'s own bank which drain_av just freed. Replaces the
                # Pool partition_all_reduce (3.8us) that stalled the DVE
                # queue head whenever the scheduler hoisted recip (v6
                # lesson).
                zbc = bank(6 + ((h - 1) & 1))
                nc.tensor.matmul(zbc, ones[:], acc[:], start=True, stop=True)
                e["zbc"] = zbc
            finish_h2(h)

        # ---------------- P3: o_proj ----------------
        for c in range(8):
            wts = wts_o0 if c == 0 else load_w_chunk(wo_t, c)
            for qt in range(4):
                ps = pp_bank()
                for a in range(NDT):
                    nc.tensor.matmul(
                        ps, aT[:, a, qt * 128:(qt + 1) * 128], wslice(wts, a),
                        start=(a == 0), stop=(a == NDT - 1),
                    )
                yt = ysb.tile([128, 512], F32, tag="yt")
                if c == 7 and qt == 3:
                    # last tile: copy+DMA in halves so the final DMA
                    # overlaps the second half's copy (shorter tail)
                    for hf in range(2):
                        sl = slice(hf * 256, (hf + 1) * 256)
                        nc.scalar.copy(out=yt[:, sl], in_=ps[:, sl])
                        nc.scalar.dma_start(
                            out=y[qt * 128:(qt + 1) * 128,
                                  c * 512 + hf * 256:c * 512 + (hf + 1) * 256],
                            in_=yt[:, sl],
                        )
                else:
                    nc.scalar.copy(out=yt[:], in_=ps)
                    nc.scalar.dma_start(
                        out=y[qt * 128:(qt + 1) * 128, c * 512:(c + 1) * 512],
                        in_=yt[:],
                    )

    nc.finalize()
    return nc


def _tile_w(wT, nchunk):
    """[4096, nchunk*512] -> [nchunk, 4, 128, 8, 512] with
    element (c, s, p, a, col) = wT[(s*8+a)*128 + p, c*512 + col]."""
    return np.ascontiguousarray(
        wT.reshape(4, 8, 128, nchunk, 512).transpose(3, 0, 2, 1, 4)
    )


def _prep_inputs(inputs):
    pos = np.asarray(inputs["positions"]).astype(np.int32)
    hs = np.asarray(inputs["hidden_states"], dtype=np.float32)
    wq = np.asarray(inputs["wq"], dtype=np.float32)
    wk = np.asarray(inputs["wk"], dtype=np.float32)
    wv = np.asarray(inputs["wv"], dtype=np.float32)
    wo = np.asarray(inputs["wo"], dtype=np.float32)
    qw = np.asarray(inputs["q_norm_w"], dtype=np.float32)
    kw = np.asarray(inputs["k_norm_w"], dtype=np.float32)

    half = HD // 2
    inv_freq = (
        1.0 / (ROPE_BASE ** (np.arange(0, half, dtype=np.float32) * 2.0 / HD))
    ).astype(np.float32)
    ang = pos.astype(np.float32)[:, None] * inv_freq[None, :]  # [S, 64]
    cos = np.cos(ang).astype(np.float32)
    sin = np.sin(ang).astype(np.float32)

    def tab(w):
        w1, w2 = w[:half][None, :], w[half:][None, :]
        return np.ascontiguousarray(
            np.concatenate([cos * w1, sin * w1, cos * w2, sin * w2], axis=1)
        ).astype(np.float32)  # [S, 256] = [cA|sA|cB|sB]

    tq = tab(qw)
    tk = tab(kw)

    wkv_t = _tile_w(np.concatenate([wk, wv], axis=0).T.astype(_BF), 4)
    wq_t = _tile_w(wq.T.astype(_BF), 8)
    wo_t = _tile_w(wo.T.astype(_BF), 8)

    in_maps = []
    for core in range(N_CORES):
        b, qh = core // 2, core % 2
        hsb = np.ascontiguousarray(hs[b].T).astype(_BF)  # [4096, 1024]
        # own 512 tokens, [a*128+p, tt*128+j] -> [tt, p, a, j]
        hsd = np.ascontiguousarray(
            hsb[:, qh * SQ:(qh + 1) * SQ]
            .reshape(NDT, 128, 4, 128).transpose(2, 1, 0, 3)
        )
        in_maps.append(
            dict(
                hs_d=hsd,
                wkv_t=wkv_t,
                wq_t=wq_t,
                wo_t=wo_t,
                ropeq=np.ascontiguousarray(tq[qh * SQ:(qh + 1) * SQ]),
                ropek=np.ascontiguousarray(tk[qh * SQ:(qh + 1) * SQ]),
            )
        )
    return in_maps


_NC_CACHE = {}


def _get_nc():
    if "nc" not in _NC_CACHE:
        _NC_CACHE["nc"] = build_bass()
    return _NC_CACHE["nc"]


def _run(inputs, **spmd_kwargs):
    nc = _get_nc()
    in_maps = _prep_inputs(inputs)
    res = run_bass_kernel_spmd(nc, in_maps, list(range(N_CORES)), **spmd_kwargs)
    out = np.empty((B, S, HIDDEN), dtype=np.float32)
    for core in range(N_CORES):
        b, qh = core // 2, core % 2
        out[b, qh * SQ:(qh + 1) * SQ, :] = res.results[core]["y"]
    return out, res


def kernel(**inputs) -> np.ndarray:
    out, _ = _run(inputs)
    return out


if __name__ == "__main__":
    nc = build_bass()
    print("built OK:", len(nc.m.functions[0].blocks), "blocks")

